# revision 22
# baseline (speedup 1.0000x reference)
"""Multi-head causal self-attention (B=32, S=512, E=768, H=12, D=64) on 8 TRN2 cores.

Sharding: pure data-parallel over batch (4 batches per core), no collectives.

Per-core layout strategy (v2 — transposed PV):
  - x is fed pre-transposed (feature-major) as xT [E, 2048tok].
  - Q^T, K^T computed feature-major per head-pair (feature tile == head pair),
    stored bf16: QT_hp = Wq[:, hp].T @ xT.
  - V computed token-major per 128-token tile with an extra all-ones column
    per head ("V_aug" [tok, H*(D+1)], bf16); the ones column rides the PV
    matmul as output row 64 and produces the softmax denominators for free.
  - scores^T[k,q] = K Q^T per (head, k-tile) with causal-trimmed q range,
    both heads of a pair packed into the PE via tile_position row groups.
  - exp() on ScalarE reads score PSUM directly (1/sqrt(D) folded into the
    scale), writes bf16; causal mask is a post-exp 0/1 multiply of the
    diagonal 128x128 block on VectorE (off the PE->ACT critical path).
  - PV is computed TRANSPOSED: ot[d, q] = sum_k V_aug[k, d] * P^T[k, q] with
    lhsT = V_aug tile (65 cols) and rhs = exp(scores^T) slices, accumulated
    per q-tile in a [65, 512] PSUM tile. Row 64 = denominators. This keeps
    the matmul moving dim large and eliminates the Y transposes entirely.
  - Normalize: reciprocal of row 64 (DVE), partition-broadcast to 64 rows
    (GpSimd), multiply rows 0-63 (DVE) writing the feature-major Y^T tile.
    Even heads land in yt[0:64, hp, :]; odd heads land in a staging tile
    yt_od[0:64, hp, :] which one SBUF->SBUF DMA per batch shifts to
    yt[64:128, :, :] (DVE cannot cross partitions; DMA can).
  - Output projection consumes yt (feature-major) as lhsT and lands
    token-major for a contiguous DMA out.
  - Emission is software-pipelined (scores of head-pair hp+1 before the PV
    block of hp; previous batch's output projection spread through the PV
    gaps; next batch's xT DMA prefetched mid-batch).
  - Projection matmuls use float32r (relaxed fp32, full PE rate at moving
    dim >= 256); attention matmuls (scores, PV) use bf16 operands (full PE
    rate at any moving dim). End-to-end absmax-relative error ~2e-3.
"""

import os
import sys

import numpy as np

for _p in ("/opt/trn_rl_repo", "/opt/trn_rl_repo/concourse"):
    if _p not in sys.path:
        sys.path.insert(0, _p)

import concourse.bass as bass
import concourse.bacc as bacc
import concourse.mybir as mybir
import concourse.tile as tile

P = 128
E = 768
S = 512
H = 12
D = 64
HP = H // 2          # head pairs
KT = E // P          # 6 feature k-tiles
N_CORES = 8
B_FULL = 32
B_CORE = B_FULL // N_CORES   # 4 batches per core
TOK = B_CORE * S             # 2048 tokens per core
ST = S // P                  # 4 token tiles per sequence
F32 = mybir.dt.float32
BF16 = mybir.dt.bfloat16

# number of 384-wide chunks for the V / O projections
CH = 2
CHW = E // CH  # 384

# dtype config (env-switchable for experiments)
_PDT_NAME = os.environ.get("BASS_PDT", "f32r")   # projection operands
_SDT_NAME = os.environ.get("BASS_SDT", "bf16")   # attention operands
_DTYPES = {"f32": F32, "f32r": mybir.dt.float32r, "bf16": BF16}
# normalization broadcast: "gpsimd" (Pool engine) or "pe" (matmul broadcast)
NORM_BCAST = os.environ.get("BASS_NORM", "gpsimd")
DBG = os.environ.get("BASS_DBG", "0") == "1"


def build_program(with_bias: bool, repeat: int = 1, hw_loop: bool = False):
    PDT = _DTYPES[_PDT_NAME]
    SDT = _DTYPES[_SDT_NAME]
    nc = bacc.Bacc(None)

    xt_d = nc.dram_tensor("xt", [E, TOK], PDT, kind="ExternalInput")
    w_d = {
        n: nc.dram_tensor(n, [E, E], PDT, kind="ExternalInput")
        for n in ("wq", "wk", "wv", "wo")
    }
    consts_d = nc.dram_tensor("consts", [P, 3 * P], F32, kind="ExternalInput")
    if with_bias:
        bqk_d = nc.dram_tensor("bqk", [P, 2 * KT], F32, kind="ExternalInput")
        bv_d = nc.dram_tensor("bvb", [P, E], F32, kind="ExternalInput")
        bo_d = nc.dram_tensor("bob", [P, E], F32, kind="ExternalInput")
    y_d = nc.dram_tensor("y", [TOK, E], F32, kind="ExternalOutput")
    dbg_d = nc.dram_tensor("dbg", [P, 4, S], F32, kind="ExternalOutput") if DBG else None

    with tile.TileContext(nc) as tc:
        with (
            tc.tile_pool(name="wpool", bufs=1) as wpool,
            tc.tile_pool(name="xpool", bufs=2) as xpool,
            tc.tile_pool(name="qkpool", bufs=int(os.environ.get("B_QK", "3"))) as qkpool,
            tc.tile_pool(name="vpool", bufs=int(os.environ.get("B_VS", "2"))) as vpool,
            tc.tile_pool(name="ppool", bufs=int(os.environ.get("B_PT", "8"))) as ppool,
            tc.tile_pool(name="mdpool", bufs=int(os.environ.get("B_MD", "6"))) as mdpool,
            tc.tile_pool(name="ytpool", bufs=2) as ytpool,
            tc.tile_pool(name="rpool", bufs=int(os.environ.get("B_R", "2"))) as rpool,
            tc.tile_pool(name="opool", bufs=2) as opool,
            tc.tile_pool(name="dbgpool", bufs=1) as dbgpool,
            tc.tile_pool(name="ps_mm", bufs=int(os.environ.get("B_MM", "2")), space="PSUM") as ps_mm,
            tc.tile_pool(name="ps_sc", bufs=int(os.environ.get("B_SC", "1")), space="PSUM") as ps_sc,
            tc.tile_pool(name="ps_ot", bufs=int(os.environ.get("B_OT", "2")), space="PSUM") as ps_ot,
        ):
            # ---- persistent constants ----
            w_sb = {}
            for n in ("wq", "wk", "wv", "wo"):
                t = wpool.tile([P, KT, E], PDT, tag=n)
                nc.sync.dma_start(t[:], w_d[n][:].rearrange("(ko ki) m -> ki ko m", ki=P))
                w_sb[n] = t
            cons = wpool.tile([P, 3 * P], F32, tag="consts")
            nc.sync.dma_start(cons[:], consts_d[:])
            mask01 = cons[:, 2 * P : 3 * P]
            # bf16 copies of the constants the attention path needs
            mask_b = wpool.tile([P, P], SDT, tag="mask_b")
            nc.any.tensor_copy(out=mask_b[:], in_=mask01)
            ones_b = wpool.tile([P, D], BF16, tag="ones_b")
            nc.gpsimd.memset(ones_b[:], 1.0)
            if NORM_BCAST == "ar":
                # zeros carriers (double-buffered over hp): reciprocal drops
                # 1/den into row 64, then one partition all-reduce (add) per
                # head pair replicates it to rows 0-64 for both heads
                zc2 = wpool.tile([D + 1, 2, 2, S], F32, tag="zc2")
                nc.gpsimd.memset(zc2[:], 0.0)
            if with_bias:
                bqk = wpool.tile([P, 2 * KT], F32, tag="bqk")
                nc.sync.dma_start(bqk[:], bqk_d[:])
                bvb = wpool.tile([P, E], F32, tag="bvb")
                nc.sync.dma_start(bvb[:], bv_d[:])
                bob = wpool.tile([P, E], F32, tag="bob")
                nc.sync.dma_start(bob[:], bo_d[:])

            xt_r = xt_d[:].rearrange("(ko ki) t -> ki ko t", ki=P)

            xts_t = {}

            def load(pos, b):
                tok0 = (b % B_CORE) * S
                xts = xpool.tile([P, KT, S], PDT, tag="xts")
                nc.sync.dma_start(xts[:], xt_r[:, :, tok0 : tok0 + S])
                xts_t[pos] = xts

            def vproj(b, xts):
                # ---- V projection (token-major, augmented with ones cols) ----
                vs = []
                for tt in range(ST):
                    v_t = vpool.tile([P, H, D + 1], SDT, tag=f"vs{tt}")
                    nc.gpsimd.memset(v_t[:, :, D : D + 1], 1.0)
                    for ch in range(CH):
                        ps = ps_mm.tile([P, S], F32, tag="mm")
                        psc = ps[:, :CHW]
                        for k in range(KT):
                            nc.tensor.matmul(
                                psc,
                                xts[:, k, tt * P : (tt + 1) * P],
                                w_sb["wv"][:, k, ch * CHW : (ch + 1) * CHW],
                                start=(k == 0),
                                stop=(k == KT - 1),
                            )
                        if with_bias:
                            nc.vector.tensor_add(
                                out=psc, in0=psc,
                                in1=bvb[:, ch * CHW : (ch + 1) * CHW],
                            )
                        hpc = CHW // D  # heads per chunk (6)
                        dst = v_t[:, ch * hpc : (ch + 1) * hpc, 0:D]
                        nc.any.tensor_copy(out=dst, in_=psc.rearrange("p (h d) -> p h d", d=D))
                    vs.append(v_t)
                return vs

            def qk_scores(b, xts, hp):
                # Q^T / K^T for this head pair (feature tile hp)
                qk = {}
                for name, tag in (("wq", "qt"), ("wk", "kt")):
                    dst = qkpool.tile([P, S], SDT, tag=tag)
                    ps = ps_mm.tile([P, S], F32, tag="mm")
                    for k in range(KT):
                        nc.tensor.matmul(
                            ps[:],
                            w_sb[name][:, k, hp * P : (hp + 1) * P],
                            xts[:, k, :],
                            start=(k == 0),
                            stop=(k == KT - 1),
                        )
                    if with_bias:
                        col = (0 if name == "wq" else KT) + hp
                        nc.vector.tensor_scalar_add(
                            dst[:], ps[:], bqk[:, col : col + 1]
                        )
                    else:
                        nc.any.tensor_copy(out=dst[:], in_=ps[:])
                    qk[tag] = dst
                qt, kt = qk["qt"], qk["kt"]

                # scores^T + exp, causal-trimmed per k-tile
                pts = []  # pts[i] = exp(scores^T) [P, 2, Nq] (heads of pair)
                for i in range(ST):
                    nq = S - i * P
                    qoff = i * P
                    ps = ps_sc.tile([P, 2, S], F32, tag="sc")
                    for hh in range(2):
                        ro = hh * D
                        nc.tensor.matmul(
                            ps[:, hh, 0:nq],
                            kt[ro : ro + D, i * P : (i + 1) * P],
                            qt[ro : ro + D, qoff:S],
                            start=True,
                            stop=True,
                            tile_position=(ro, 0),
                        )
                    pt = ppool.tile([P, 2, S], SDT, tag="pt")
                    nc.scalar.activation(
                        pt[:, :, 0:nq],
                        ps[:, :, 0:nq],
                        mybir.ActivationFunctionType.Exp,
                        scale=0.125,
                    )
                    # causal mask: zero the upper triangle of the diagonal
                    # block, off the PE->ACT critical path (VectorE, post-exp)
                    md = mdpool.tile([P, 2, P], SDT, tag="md")
                    nc.vector.tensor_mul(
                        out=md[:], in0=pt[:, :, 0:P],
                        in1=mask_b[:, None, :].to_broadcast((P, 2, P)),
                    )
                    pts.append((pt, md))
                return pts

            def dbg_dump(slot, src_ap, row0, nrows):
                t = dbgpool.tile([P, S], F32, tag=f"dbg{slot}")
                nc.any.tensor_copy(out=t[row0 : row0 + nrows, :], in_=src_ap)
                nc.sync.dma_start(dbg_d[row0 : row0 + nrows, slot, :], t[row0 : row0 + nrows, :])

            def pv_block(hp, pts, vs, yt, yt_od, dbg=False):
                # ot[d, q] = sum_k V_aug[k, d] P^T[k, q], accumulated per
                # q-tile j over k-tiles i<=j.  Row 64 = softmax denominators.
                RDT = BF16 if NORM_BCAST == "pe" else F32
                if NORM_BCAST != "ar":
                    r = rpool.tile([P, 2, S], RDT, tag="r")
                if NORM_BCAST in ("gpsimd", "dma"):
                    rb = rpool.tile([D, 2, S], F32, tag="rb")
                elif NORM_BCAST == "ar":
                    rb = rpool.tile([D + 1, 2, S], F32, tag="rb")
                else:
                    rb = None
                ots = []
                for hh in range(2):
                    h = 2 * hp + hh
                    ot = ps_ot.tile([D + 1, S], F32, tag=f"ot{hh}")
                    for j in range(ST):
                        for i in range(j + 1):
                            pt, md = pts[i]
                            rhs = (
                                md[:, hh, :]
                                if i == j
                                else pt[:, hh, (j - i) * P : (j - i + 1) * P]
                            )
                            nc.tensor.matmul(
                                ot[:, j * P : (j + 1) * P],
                                vs[i][:, h, :],
                                rhs,
                                start=(i == 0),
                                stop=(i == j),
                            )
                    ots.append(ot)
                    if NORM_BCAST == "ar":
                        zc = zc2[:, hp % 2, :, :]
                        nc.vector.reciprocal(zc[D : D + 1, hh, :], ot[D : D + 1, :])
                        if hh == 1:
                            import concourse.bass_isa as bass_isa
                            nc.gpsimd.partition_all_reduce(
                                rb[:].rearrange("p hh s -> p (hh s)"),
                                zc.rearrange("p hh s -> p (hh s)"),
                                channels=D + 1,
                                reduce_op=bass_isa.ReduceOp.add,
                            )
                            for h2 in range(2):
                                nc.vector.tensor_mul(
                                    out=(yt[0:D, hp, :] if h2 == 0 else yt_od[0:D, hp, :]),
                                    in0=ots[h2][0:D, :], in1=rb[0:D, h2, :],
                                )
                        continue
                    nc.vector.reciprocal(r[D : D + 1, hh, :], ot[D : D + 1, :])
                    if NORM_BCAST == "gpsimd":
                        nc.gpsimd.partition_broadcast(
                            rb[:, hh, :], r[D : D + 1, hh, :]
                        )
                        rbs = rb[:, hh, :]
                    elif NORM_BCAST == "dma":
                        # broadcast by DMA: descriptors re-read the same
                        # source row for every destination partition
                        nc.sync.dma_start(
                            rb[:, hh, :],
                            r[D : D + 1, hh, :].to_broadcast((D, S)),
                        )
                        rbs = rb[:, hh, :]
                    else:
                        rb_ps = ps_mm.tile([P, S], F32, tag="mm")
                        nc.tensor.matmul(
                            rb_ps[0:D, :],
                            ones_b[D : D + 1, :],
                            r[D : D + 1, hh, :],
                            start=True,
                            stop=True,
                        )
                        rbs = rb_ps[0:D, :]
                    dst = yt[0:D, hp, :] if hh == 0 else yt_od[0:D, hp, :]
                    if dbg and hh == 0:
                        dbg_dump(0, ot[:], 0, D + 1)
                        dbg_dump(1, r[D : D + 1, hh, :], D, 1)
                        dbg_dump(2, rbs, 0, D)
                    nc.vector.tensor_mul(out=dst, in0=ot[0:D, :], in1=rbs)

            def oproj_tt(b, yt, tt):
                tok0 = (b % B_CORE) * S
                o_sb = opool.tile([P, E], F32, tag="osb")
                for ch in range(CH):
                    ps = ps_mm.tile([P, S], F32, tag="mm")
                    psc = ps[:, :CHW]
                    for k in range(KT):
                        nc.tensor.matmul(
                            psc,
                            yt[:, k, tt * P : (tt + 1) * P],
                            w_sb["wo"][:, k, ch * CHW : (ch + 1) * CHW],
                            start=(k == 0),
                            stop=(k == KT - 1),
                        )
                    nc.any.tensor_copy(
                        out=o_sb[:, ch * CHW : (ch + 1) * CHW], in_=psc
                    )
                if with_bias:
                    nc.vector.tensor_add(out=o_sb[:], in0=o_sb[:], in1=bob[:])
                nc.sync.dma_start(
                    y_d[tok0 + tt * P : tok0 + (tt + 1) * P, :], o_sb[:]
                )

            def run_batches(batches):
                # Software-pipelined emission: scores of head-pair hp+1 are
                # emitted before the PV block of hp, so the tensor engine's
                # in-order stream always has matmuls to run while the
                # exp(ACT) -> normalize(DVE) chains drain.
                load(0, batches[0])
                pending_o = None  # (b, yt) of the previous batch
                for idx, b in enumerate(batches):
                    xts = xts_t.pop(idx)
                    vs = vproj(b, xts)
                    yt = ytpool.tile([P, KT, S], PDT, tag="yt")
                    yt_od = ytpool.tile([D, KT, S], PDT, tag="ytod")
                    pts_next = qk_scores(b, xts, 0)
                    for hp in range(HP):
                        pts_cur = pts_next
                        # previous batch's output projection, one token tile
                        # at a time, spread through the PV chain gaps
                        if pending_o is not None and hp < ST:
                            oproj_tt(*pending_o, hp)
                        if hp == 2 and idx + 1 < len(batches):
                            load(idx + 1, batches[idx + 1])
                        if hp + 1 < HP:
                            pts_next = qk_scores(b, xts, hp + 1)
                        pv_block(hp, pts_cur, vs, yt, yt_od,
                                 dbg=DBG and idx == 0 and hp == 0)
                    # odd heads: shift partitions 0-63 -> 64-127 (DMA can
                    # cross partitions; the vector engines cannot)
                    nc.sync.dma_start(yt[D:P, :, :], yt_od[:])
                    if DBG and idx == 0:
                        dbg_dump(3, yt[:, 0, :], 0, P)
                    pending_o = (b, yt)
                for tt in range(ST):
                    oproj_tt(*pending_o, tt)

            if hw_loop and repeat > 1:
                with tc.For_i(0, repeat, 1):
                    run_batches(list(range(B_CORE)))
            else:
                run_batches([b % B_CORE for b in range(B_CORE * repeat)])

    nc.compile()
    return nc


def _host_consts():
    ident = np.eye(P, dtype=np.float32)
    k_idx = np.arange(P, dtype=np.int64)[:, None]
    q_idx = np.arange(P, dtype=np.int64)[None, :]
    maskb = np.where(k_idx <= q_idx, 0.0, -1.0e6).astype(np.float32)
    mask01 = (k_idx <= q_idx).astype(np.float32)
    return np.concatenate([ident, maskb, mask01], axis=1)  # [P, 3P]


_PROG_CACHE = {}


def _get_program(with_bias: bool):
    if with_bias not in _PROG_CACHE:
        _PROG_CACHE[with_bias] = build_program(with_bias)
    return _PROG_CACHE[with_bias]


def _np_dt(name):
    if name == "bf16":
        import ml_dtypes
        return np.dtype(ml_dtypes.bfloat16)
    return np.dtype(np.float32)


def make_in_maps(x, Wq, bq, Wk, bk, Wv, bv, Wo, bo, with_bias):
    consts = _host_consts()
    pdt = _np_dt(_PDT_NAME)
    maps = []
    for c in range(N_CORES):
        xc = np.ascontiguousarray(
            x[c * B_CORE : (c + 1) * B_CORE]  # [B_CORE, S, E]
            .reshape(TOK, E)
            .T  # [E, TOK]
        ).astype(pdt)
        m = {
            "xt": xc,
            "wq": np.ascontiguousarray(Wq).astype(pdt),
            "wk": np.ascontiguousarray(Wk).astype(pdt),
            "wv": np.ascontiguousarray(Wv).astype(pdt),
            "wo": np.ascontiguousarray(Wo).astype(pdt),
            "consts": consts,
        }
        if with_bias:
            bqk = np.concatenate(
                [np.asarray(bq).reshape(KT, P).T, np.asarray(bk).reshape(KT, P).T],
                axis=1,
            ).astype(np.float32)
            m["bqk"] = np.ascontiguousarray(bqk)
            m["bvb"] = np.ascontiguousarray(
                np.broadcast_to(np.asarray(bv, dtype=np.float32), (P, E))
            )
            m["bob"] = np.ascontiguousarray(
                np.broadcast_to(np.asarray(bo, dtype=np.float32), (P, E))
            )
        maps.append(m)
    return maps


def kernel(x, Wq, bq, Wk, bk, Wv, bv, Wo, bo):
    from concourse.bass_utils import run_bass_kernel_spmd

    x = np.asarray(x, dtype=np.float32)
    with_bias = any(
        float(np.abs(np.asarray(b)).max()) != 0.0 for b in (bq, bk, bv, bo)
    )
    nc = _get_program(with_bias)
    in_maps = make_in_maps(x, Wq, bq, Wk, bk, Wv, bv, Wo, bo, with_bias)
    res = run_bass_kernel_spmd(nc, in_maps, core_ids=list(range(N_CORES)))
    out = np.empty((B_FULL, S, E), dtype=np.float32)
    for c in range(N_CORES):
        out[c * B_CORE : (c + 1) * B_CORE] = res.results[c]["y"].reshape(B_CORE, S, E)
    return out
